# revision 4
# baseline (speedup 1.0000x reference)
"""DGM layer (encoder + per-graph pairwise-distance gumbel top-k edges) on 8 trn2 cores.

Contract: kernel(**inputs) takes FULL inputs (x[32768,128], W[128,64], b[64],
temperature[]) and returns (out[32768,64] f32, edges_idx[2, 32*1024*16] int32),
matching reference.py. Sharding: data-parallel over graphs, 4 graphs/core.
"""

import os
import numpy as np

import concourse.bacc as bacc
import concourse.tile as tile
import concourse.mybir as mybir
import concourse.bass as bass
from concourse.bass import ts
from concourse.bass_utils import run_bass_kernel_spmd
from concourse.masks import make_identity

B = 32
N = 1024          # nodes per graph
C = 64            # embed dim
K = 16            # top-k
D_IN = 128
NCORES = 8
GPC = B // NCORES  # graphs per core
ROWS = GPC * N     # rows per core (4096)
NT = N // 128      # row-tiles per graph (8)
NEG_BIG = -3.0e38

_cache = {}


def _gumbel_z():
    """z = jax.random.gumbel(key(1), (B, N, N), f32) - fixed constant of the problem.

    Generated on host CPU (threefry is platform-deterministic)."""
    if "z" in _cache:
        return _cache["z"]
    import jax
    with jax.default_device(jax.devices("cpu")[0]):
        z = jax.random.gumbel(jax.random.key(1), (B, N, N), np.float32)
        z = np.asarray(z)
    _cache["z"] = z
    return z


def _build_nc(reps=1):
    """Build the per-core Bass program. All 8 cores run the same NEFF on their
    own shard. reps>1 repeats the whole compute for timing measurements."""
    nc = bacc.Bacc("TRN2", target_bir_lowering=False, debug=False, num_devices=NCORES)
    f32 = mybir.dt.float32
    u16 = mybir.dt.uint16

    x_d = nc.dram_tensor("x_s", [ROWS, D_IN], f32, kind="ExternalInput")
    z_d = nc.dram_tensor("z_s", [ROWS, N], f32, kind="ExternalInput")
    w_d = nc.dram_tensor("w_in", [D_IN, C], f32, kind="ExternalInput")
    b_d = nc.dram_tensor("b_in", [C, 1], f32, kind="ExternalInput")
    t_d = nc.dram_tensor("t_in", [1, 1], f32, kind="ExternalInput")
    out_d = nc.dram_tensor("out_s", [ROWS, C], f32, kind="ExternalOutput")
    idx_d = nc.dram_tensor("idx_s", [ROWS, K], u16, kind="ExternalOutput")

    with tile.TileContext(nc) as tc:
        with (
            tc.tile_pool(name="const", bufs=1) as constp,
            tc.tile_pool(name="graph", bufs=2) as graphp,
            tc.tile_pool(name="work", bufs=3) as workp,
            tc.tile_pool(name="psum", bufs=3, space="PSUM") as psump,
            tc.tile_pool(name="psum_small", bufs=3, space="PSUM") as psump_s,
        ):
            # ---- constants ----
            ident = constp.tile([128, 128], f32)
            make_identity(nc, ident[:])
            w_sb = constp.tile([D_IN, C], f32)
            nc.sync.dma_start(w_sb[:], w_d.ap())
            b_sb = constp.tile([C, 1], f32)
            nc.sync.dma_start(b_sb[:], b_d.ap())
            t_sb = constp.tile([1, 1], f32)
            nc.sync.dma_start(t_sb[:], t_d.ap())
            ones_row = constp.tile([1, 128], f32)
            nc.vector.memset(ones_row[:], 1.0)
            ones_col = constp.tile([C, 1], f32)
            nc.vector.memset(ones_col[:], 1.0)

            # broadcast temperature to [128,1]: psum = ones_row.T @ t_sb
            tb_ps = psump_s.tile([128, 1], f32, tag="mm")
            nc.tensor.matmul(tb_ps[:], ones_row[:], t_sb[:])
            two_t = constp.tile([128, 1], f32)
            nc.scalar.activation(two_t[:], tb_ps[:],
                                 mybir.ActivationFunctionType.Identity, scale=2.0)
            neg_t = constp.tile([128, 1], f32)
            nc.scalar.activation(neg_t[:], tb_ps[:],
                                 mybir.ActivationFunctionType.Identity, scale=-1.0)

            for _rep in range(reps):
                for g in range(GPC):
                    # ---- per-graph prep ----
                    # xT[g]: [128 (k), 1024 (m)] via PE transpose of 8 [128,128] tiles
                    xT = graphp.tile([128, N], f32, tag="xT")
                    for t in range(NT):
                        xt_in = workp.tile([128, D_IN], f32, tag="xtin")
                        nc.sync.dma_start(xt_in[:], x_d.ap()[ts(g * NT + t, 128), :])
                        tp_ps = psump_s.tile([128, 128], f32, tag="mm")
                        nc.tensor.transpose(tp_ps[:], xt_in[:], ident[:])
                        nc.scalar.copy(xT[:, ts(t, 128)], tp_ps[:])

                    # hT = W^T @ xT + b : [64, 1024]
                    hT = graphp.tile([C, N], f32, tag="hT")
                    h2 = graphp.tile([C, N], f32, tag="h2")
                    for jc in range(2):
                        h_ps = psump_s.tile([C, 512], f32, tag="mm")
                        nc.tensor.matmul(h_ps[:], w_sb[:], xT[:, ts(jc, 512)])
                        nc.scalar.add(hT[:, ts(jc, 512)], h_ps[:], b_sb[:])
                        nc.scalar.square(h2[:, ts(jc, 512)], hT[:, ts(jc, 512)])
                    # wrow = -0.5 * colsum(h2): [1, 1024]
                    wrow = graphp.tile([1, N], f32, tag="wrow")
                    for jc in range(2):
                        sq_ps = psump_s.tile([1, 512], f32, tag="mm")
                        nc.tensor.matmul(sq_ps[:], ones_col[:], h2[:, ts(jc, 512)])
                        nc.scalar.activation(wrow[:, ts(jc, 512)], sq_ps[:],
                                             mybir.ActivationFunctionType.Identity,
                                             scale=-0.5)

                    # ---- per row-tile ----
                    for t in range(NT):
                        rows = ts(g * NT + t, 128)
                        # encoder rows out: transpose hT chunk -> [128, 64]
                        enc_ps = psump_s.tile([128, C], f32, tag="mm")
                        nc.tensor.transpose(enc_ps[:], hT[:, ts(t, 128)],
                                            ident[:C, :C])
                        out_sb = workp.tile([128, C], f32, tag="outsb")
                        nc.scalar.copy(out_sb[:], enc_ps[:])
                        nc.sync.dma_start(out_d.ap()[rows, :], out_sb[:])
                        # sq_i column + exp bias
                        sq_i = workp.tile([128, 1], f32, tag="sqi")
                        sq_sc = workp.tile([128, C], f32, tag="sqsc")
                        nc.scalar.activation(sq_sc[:], enc_ps[:],
                                             mybir.ActivationFunctionType.Square,
                                             accum_out=sq_i[:])
                        nbias = workp.tile([128, 1], f32, tag="nbias")
                        nc.vector.tensor_scalar_mul(nbias[:], sq_i[:], neg_t[:])

                        # z prefetch
                        z_sb = workp.tile([128, N], f32, tag="z")
                        nc.sync.dma_start(z_sb[:], z_d.ap()[rows, :])

                        # gram + exp -> s
                        s_sb = workp.tile([128, N], f32, tag="s")
                        for jc in range(2):
                            g_ps = psump.tile([128, 512], f32, tag="gps")
                            nc.tensor.matmul(g_ps[:], hT[:, ts(t, 128)],
                                             hT[:, ts(jc, 512)],
                                             start=True, stop=False)
                            nc.tensor.matmul(g_ps[:], ones_row[:],
                                             wrow[:, ts(jc, 512)],
                                             start=False, stop=True)
                            nc.scalar.activation(s_sb[:, ts(jc, 512)], g_ps[:],
                                                 mybir.ActivationFunctionType.Exp,
                                                 bias=nbias[:], scale=two_t[:])
                        nc.vector.tensor_add(s_sb[:], s_sb[:], z_sb[:])

                        # top-16: two max8 rounds
                        idx_sb = workp.tile([128, K], u16, tag="idx")
                        v8a = workp.tile([128, 8], f32, tag="v8a")
                        v8b = workp.tile([128, 8], f32, tag="v8b")
                        nc.vector.max(v8a[:], s_sb[:])
                        nc.vector.max_index(idx_sb[:, 0:8], v8a[:], s_sb[:])
                        nc.vector.match_replace(s_sb[:], v8a[:], s_sb[:], NEG_BIG)
                        nc.vector.max(v8b[:], s_sb[:])
                        nc.vector.max_index(idx_sb[:, 8:16], v8b[:], s_sb[:])
                        nc.sync.dma_start(idx_d.ap()[rows, :], idx_sb[:])

    nc.compile()
    return nc


def _get_nc(reps=1):
    key = ("nc", reps)
    if key not in _cache:
        _cache[key] = _build_nc(reps)
    return _cache[key]


def _shard_inputs(x, W, b, temperature):
    z = _gumbel_z().reshape(B * N, N)
    x = np.ascontiguousarray(np.asarray(x, np.float32))
    in_maps = []
    for c in range(NCORES):
        sl = slice(c * ROWS, (c + 1) * ROWS)
        in_maps.append({
            "x_s": x[sl],
            "z_s": np.ascontiguousarray(z[sl]),
            "w_in": np.ascontiguousarray(np.asarray(W, np.float32)),
            "b_in": np.ascontiguousarray(np.asarray(b, np.float32).reshape(C, 1)),
            "t_in": np.asarray(temperature, np.float32).reshape(1, 1),
        })
    return in_maps


def _assemble(results):
    out = np.concatenate([r["out_s"] for r in results], axis=0)
    idx = np.concatenate([r["idx_s"] for r in results], axis=0)  # [32768, 16] u16
    cols_local = idx.astype(np.int32).reshape(B, N, K)
    offsets = (np.arange(B, dtype=np.int32) * N)[:, None, None]
    rows = np.broadcast_to(
        np.arange(N, dtype=np.int32)[None, :, None] + offsets, (B, N, K))
    cols = cols_local + offsets
    edges_idx = np.stack((rows.reshape(-1), cols.reshape(-1)))
    return out, edges_idx


def kernel(x, W, b, temperature):
    nc = _get_nc()
    in_maps = _shard_inputs(x, W, b, temperature)
    res = run_bass_kernel_spmd(nc, in_maps, core_ids=list(range(NCORES)))
    return _assemble(res.results)


# revision 7
# speedup vs baseline: 617.7363x; 617.7363x over previous
"""DGM layer (encoder + per-graph pairwise-distance gumbel top-k edges) on 8 trn2 cores.

Contract: kernel(**inputs) takes FULL inputs (x[32768,128], W[128,64], b[64],
temperature[]) and returns (out[32768,64] f32, edges_idx[2, 32*1024*16] int32),
matching reference.py. Sharding: data-parallel over graphs, 4 graphs/core.

Perf-critical structure decisions (measured on HW):
- dependent back-to-back ops on one engine stall ~10us on wait wake-up, so
  all per-row-tile work is emitted phase-interleaved (quads of tiles).
- fp32 PE matmuls are weight-load-bound (K*M cycles each, no fast path), so
  x is supplied pre-transposed by the host (no PE transposes), and the
  encoder output is streamed out directly from hT (column-major; host
  transposes it back).
- sq_i per row-tile comes from tiny PE transposes of the wrow row.
"""

import numpy as np

import concourse.bacc as bacc
import concourse.tile as tile
import concourse.mybir as mybir
from concourse.bass import ts
from concourse.bass_utils import run_bass_kernel_spmd

B = 32
N = 1024          # nodes per graph
C = 64            # embed dim
K = 16            # top-k
D_IN = 128
NCORES = 8
GPC = B // NCORES  # graphs per core
ROWS = GPC * N     # rows per core (4096)
NT = N // 128      # row-tiles per graph (8)
QUAD = 4           # row-tiles interleaved per phase group
NEG_BIG = -3.0e38
AF = mybir.ActivationFunctionType

_cache = {}


def _gumbel_z():
    """z = jax.random.gumbel(key(1), (B, N, N), f32) - fixed constant of the
    problem (threefry bits are platform-deterministic); generated on host CPU."""
    if "z" in _cache:
        return _cache["z"]
    import jax
    with jax.default_device(jax.devices("cpu")[0]):
        z = np.asarray(jax.random.gumbel(jax.random.key(1), (B, N, N), np.float32))
    _cache["z"] = z
    return z


def _build_nc(reps=1, use_f32r=False):
    nc = bacc.Bacc("TRN2", target_bir_lowering=False, debug=False, num_devices=NCORES)
    f32 = mybir.dt.float32
    f32r = mybir.dt.float32r
    u16 = mybir.dt.uint16
    gram_dt = f32r if use_f32r else f32

    xt_d = nc.dram_tensor("xT_s", [D_IN, ROWS], f32, kind="ExternalInput")
    z_d = nc.dram_tensor("z_s", [ROWS, N], f32, kind="ExternalInput")
    w_d = nc.dram_tensor("w_in", [D_IN, C], f32, kind="ExternalInput")
    b_d = nc.dram_tensor("b_in", [C, 1], f32, kind="ExternalInput")
    t_d = nc.dram_tensor("t_in", [1, 1], f32, kind="ExternalInput")
    outt_d = nc.dram_tensor("outT_s", [C, ROWS], f32, kind="ExternalOutput")
    idx_d = nc.dram_tensor("idx_s", [ROWS, K], u16, kind="ExternalOutput")

    with tile.TileContext(nc) as tc:
        with (
            tc.tile_pool(name="const", bufs=1) as constp,
            tc.tile_pool(name="graph", bufs=2) as graphp,
            tc.tile_pool(name="sbig", bufs=2 * QUAD) as sbigp,
            tc.tile_pool(name="zpool", bufs=6) as zp,
            tc.tile_pool(name="small", bufs=2 * QUAD) as smallp,
            tc.tile_pool(name="psum", bufs=4, space="PSUM") as psump,
            tc.tile_pool(name="psum2", bufs=3, space="PSUM") as psump_s,
        ):
            # ---- constants ----
            w_sb = constp.tile([D_IN, C], f32)
            nc.sync.dma_start(w_sb[:], w_d.ap())
            b_sb = constp.tile([C, 1], f32)
            nc.sync.dma_start(b_sb[:], b_d.ap())
            t_sb = constp.tile([1, 1], f32)
            nc.sync.dma_start(t_sb[:], t_d.ap())
            ones_row = constp.tile([1, 128], f32)
            nc.vector.memset(ones_row[:], 1.0)
            ones_col = constp.tile([C, 1], f32)
            nc.gpsimd.memset(ones_col[:], 1.0)
            one_one = constp.tile([1, 1], f32)
            nc.gpsimd.memset(one_one[:], 1.0)

            tb_ps = psump_s.tile([128, 1], f32, tag="mm")
            nc.tensor.matmul(tb_ps[:], ones_row[:], t_sb[:])
            two_t = constp.tile([128, 1], f32)
            nc.scalar.activation(two_t[:], tb_ps[:], AF.Identity, scale=2.0)

            for _rep in range(reps):
                # ---- per-graph prep, interleaved across all graphs ----
                xTs, hTs, wrows = {}, {}, {}
                for g in range(GPC):
                    xT = graphp.tile([D_IN, N], f32, tag="xT%d" % g, name="xT%d" % g)
                    nc.sync.dma_start(xT[:], xt_d.ap()[:, ts(g, N)])
                    xTs[g] = xT
                h_pss = {}
                for g in range(GPC):
                    hTs[g] = graphp.tile([C, N], f32, tag="hT%d" % g, name="hT%d" % g)
                    for jc in range(2):
                        h_ps = psump.tile([C, 512], f32, tag="gps", name="hps")
                        nc.tensor.matmul(h_ps[:], w_sb[:], xTs[g][:, ts(jc, 512)])
                        h_pss[(g, jc)] = h_ps
                for g in range(GPC):
                    for jc in range(2):
                        nc.scalar.add(hTs[g][:, ts(jc, 512)], h_pss[(g, jc)][:],
                                      b_sb[:])
                h2s = {}
                for g in range(GPC):
                    h2 = graphp.tile([C, N], f32, tag="h2", name="h2")
                    for jc in range(2):
                        nc.scalar.square(h2[:, ts(jc, 512)], hTs[g][:, ts(jc, 512)])
                    h2s[g] = h2
                    # stream encoder output (column-major) straight from hT
                    nc.sync.dma_start(outt_d.ap()[:, ts(g, N)], hTs[g][:])
                sq_pss = {}
                for g in range(GPC):
                    for jc in range(2):
                        sq_ps = psump_s.tile([1, 512], f32, tag="mm", name="sqps")
                        nc.tensor.matmul(sq_ps[:], ones_col[:], h2s[g][:, ts(jc, 512)])
                        sq_pss[(g, jc)] = sq_ps
                for g in range(GPC):
                    wrow = graphp.tile([1, N], f32, tag="wrow%d" % g, name="wrow%d" % g)
                    for jc in range(2):
                        nc.scalar.activation(wrow[:, ts(jc, 512)],
                                             sq_pss[(g, jc)][:], AF.Identity,
                                             scale=-0.5)
                    wrows[g] = wrow
                if use_f32r:
                    hTrs = {}
                    for g in range(GPC):
                        hTr = graphp.tile([C, N], f32r, tag="hTr%d" % g, name="hTr%d" % g)
                        nc.vector.tensor_copy(hTr[:], hTs[g][:])
                        hTrs[g] = hTr
                    gram_in = hTrs
                else:
                    gram_in = hTs

                # ---- row-tiles, phase-interleaved in quads across graphs ----
                # order: (g0,t0) (g1,t0) (g2,t0) (g3,t0) (g0,t1) ... so quad
                # members come from different graphs
                tiles = [(g, t) for t in range(NT) for g in range(GPC)]
                for q0 in range(0, len(tiles), QUAD):
                    quad = tiles[q0:q0 + QUAD]
                    # nbias = -T*sq_i = 2T * wrow_col: tiny PE transpose of wrow
                    wr_pss = {}
                    for (g, t) in quad:
                        wr_ps = psump_s.tile([128, 1], f32, tag="mm", name="wrps")
                        nc.tensor.matmul(wr_ps[:], wrows[g][:, ts(t, 128)],
                                         one_one[:])
                        wr_pss[(g, t)] = wr_ps
                    nbiases = {}
                    for (g, t) in quad:
                        nbias = smallp.tile([128, 1], f32, tag="nbias", name="nbias")
                        nc.scalar.activation(nbias[:], wr_pss[(g, t)][:],
                                             AF.Identity, scale=two_t[:])
                        nbiases[(g, t)] = nbias

                    z_sbs = {}
                    for (g, t) in quad:
                        z_sb = zp.tile([128, N], f32, tag="z", name="zsb")
                        nc.sync.dma_start(z_sb[:],
                                          z_d.ap()[ts(g * NT + t, 128), :])
                        z_sbs[(g, t)] = z_sb

                    g_pss = {}
                    for (g, t) in quad:
                        for jc in range(2):
                            g_ps = psump.tile([128, 512], f32, tag="gps", name="gps")
                            nc.tensor.matmul(g_ps[:], gram_in[g][:, ts(t, 128)],
                                             gram_in[g][:, ts(jc, 512)],
                                             start=True, stop=False)
                            nc.tensor.matmul(g_ps[:], ones_row[:],
                                             wrows[g][:, ts(jc, 512)],
                                             start=False, stop=True)
                            g_pss[(g, t, jc)] = g_ps
                    s_sbs = {}
                    for (g, t) in quad:
                        s_sbs[(g, t)] = sbigp.tile([128, N], f32, tag="s", name="ssb")
                    for jc in range(2):
                        for (g, t) in quad:
                            nc.scalar.activation(s_sbs[(g, t)][:, ts(jc, 512)],
                                                 g_pss[(g, t, jc)][:], AF.Exp,
                                                 bias=nbiases[(g, t)][:],
                                                 scale=two_t[:])
                    for i, (g, t) in enumerate(quad):
                        eng = nc.gpsimd if i % 2 == 0 else nc.vector
                        eng.tensor_add(s_sbs[(g, t)][:], s_sbs[(g, t)][:],
                                       z_sbs[(g, t)][:])

                    v8as, v8bs, idx_sbs = {}, {}, {}
                    for gt in quad:
                        v8as[gt] = smallp.tile([128, 8], f32, tag="v8a", name="v8a")
                        v8bs[gt] = smallp.tile([128, 8], f32, tag="v8b", name="v8b")
                        idx_sbs[gt] = smallp.tile([128, K], u16, tag="idx", name="idx")
                    for gt in quad:
                        nc.vector.max(v8as[gt][:], s_sbs[gt][:])
                    for gt in quad:
                        nc.vector.max_index(idx_sbs[gt][:, 0:8], v8as[gt][:],
                                            s_sbs[gt][:])
                    for gt in quad:
                        nc.vector.match_replace(s_sbs[gt][:], v8as[gt][:],
                                                s_sbs[gt][:], NEG_BIG)
                    for gt in quad:
                        nc.vector.max(v8bs[gt][:], s_sbs[gt][:])
                    for gt in quad:
                        nc.vector.max_index(idx_sbs[gt][:, 8:16], v8bs[gt][:],
                                            s_sbs[gt][:])
                    for (g, t) in quad:
                        nc.sync.dma_start(idx_d.ap()[ts(g * NT + t, 128), :],
                                          idx_sbs[(g, t)][:])

    nc.compile()
    return nc


def _get_nc(reps=1, use_f32r=False):
    key = ("nc", reps, use_f32r)
    if key not in _cache:
        _cache[key] = _build_nc(reps, use_f32r)
    return _cache[key]


def _shard_inputs(x, W, b, temperature):
    z = _gumbel_z().reshape(B * N, N)
    x = np.asarray(x, np.float32)
    in_maps = []
    for c in range(NCORES):
        sl = slice(c * ROWS, (c + 1) * ROWS)
        in_maps.append({
            "xT_s": np.ascontiguousarray(x[sl].T),
            "z_s": np.ascontiguousarray(z[sl]),
            "w_in": np.ascontiguousarray(np.asarray(W, np.float32)),
            "b_in": np.ascontiguousarray(np.asarray(b, np.float32).reshape(C, 1)),
            "t_in": np.asarray(temperature, np.float32).reshape(1, 1),
        })
    return in_maps


def _assemble(results):
    out = np.concatenate([r["outT_s"].T for r in results], axis=0)
    idx = np.concatenate([r["idx_s"] for r in results], axis=0)  # [32768, 16] u16
    cols_local = idx.astype(np.int32).reshape(B, N, K)
    offsets = (np.arange(B, dtype=np.int32) * N)[:, None, None]
    rows = np.broadcast_to(
        np.arange(N, dtype=np.int32)[None, :, None] + offsets, (B, N, K))
    cols = cols_local + offsets
    edges_idx = np.stack((rows.reshape(-1), cols.reshape(-1)))
    return out, edges_idx


def kernel(x, W, b, temperature):
    nc = _get_nc()
    in_maps = _shard_inputs(x, W, b, temperature)
    res = run_bass_kernel_spmd(nc, in_maps, core_ids=list(range(NCORES)))
    return _assemble(res.results)
